# revision 58
# baseline (speedup 1.0000x reference)
"""Causal multi-head attention forward (B=2, T=2048, C=1024, H=16, D=64)
for 8 Trainium2 NeuronCores.

Sharding: core = (batch b, head-group hg) with b in {0,1}, hg in {0..3};
each core computes QKV projection for its 4 heads on its batch, causal
flash attention for those heads, and a partial output projection
(contraction over its 256 head-feature rows of W_o). Host sums the 4
partials per batch and adds b_o.

Schedule (v2): projection, attention and o_proj are interleaved per
query chunk so the PE never drains while ScalarE runs the big exp
stream (keeps the HAM clock-gate released):

  proj(c0); for qc: [attention(qc) pairs, with proj(c{qc+1}) and
  o_proj(qc-1) groups used as PE fillers between pairs], o_proj(qc)

Per kb-pair: S matmuls are emitted h2-adjacent so the two K=64 matmuls
occupy disjoint PE row groups (tile_position (0,0)/(64,0)) and run
concurrently; one exp per (pair, h2) covers [jj0:1024] and PV matmuls
are column-trimmed to the causally live range (no es memsets needed).
PV for pair n is emitted after S of pair n+1 (lag 1) so the PE has
dep-free work while ScalarE exps pair n.

Kernel-internal layouts (per core):
  xT    [C, T]    bf16   x transposed (host-prepped)
  wqk   [C, 512]  bf16   [q cols heads0..3 | k cols heads0..3], q
                         pre-scaled by 1/sqrt(D) host-side
  wv    [C, 256]  bf16
  wo    [256, CO] bf16   W_o rows for this head group
  qkT   [512, T]  bf16   biases added at eviction (DVE, per-partition)
  S^T   [ki, qi]  PSUM   scores transposed, 2 kb blocks per tile
  expS  [ki, qi]  bf16   exp on ScalarE; causal mask via gpsimd
                         affine_select on diagonal 128-squares only
  yT'   [128, qi] PSUM   [ones|pad|v]^T @ expS -> row 0 = denominator,
                         rows 64..127 = unnormalized y^T
  norm: recip(l) -> gpsimd partition_broadcast -> DVE multiply
  out   [T, CO]   bf16   partial o_proj; host sums partials in fp32
"""

import os
import sys
from collections import deque
from contextlib import ExitStack
from dataclasses import dataclass

import numpy as np

for _p in ("/opt/trn_rl_repo",):
    if _p not in sys.path and os.path.isdir(_p):
        sys.path.insert(0, _p)

import ml_dtypes

import concourse.bass as bass
import concourse.bacc as bacc
import concourse.mybir as mybir
import concourse.tile as tile


def _install_axon_ntff_hook():
    """Provide antenv.axon_hooks (absent on this image) so bass_utils'
    trace path works; registers the ctypes NTFF hook when available."""
    import types

    if "antenv.axon_hooks" not in sys.modules:
        import antenv

        mod = types.ModuleType("antenv.axon_hooks")
        _reg = [None]
        mod.get_axon_ntff_profile_hook = lambda: _reg[0]
        mod.set_axon_ntff_profile_hook = lambda h: _reg.__setitem__(0, h)
        sys.modules["antenv.axon_hooks"] = mod
        antenv.axon_hooks = mod
    hooks = sys.modules["antenv.axon_hooks"]
    if hooks.get_axon_ntff_profile_hook() is not None:
        return
    try:
        import contextlib
        import ctypes

        lib = ctypes.CDLL("/opt/axon/libaxon_pjrt.so")
        if not hasattr(lib, "axon_start_nrt_profile"):
            return
        lib.axon_start_nrt_profile.argtypes = [
            ctypes.POINTER(ctypes.c_int64), ctypes.c_size_t]
        lib.axon_start_nrt_profile.restype = ctypes.c_int64
        lib.axon_stop_nrt_profile.argtypes = [ctypes.c_char_p]
        lib.axon_stop_nrt_profile.restype = ctypes.c_int64

        @contextlib.contextmanager
        def _hook(output_dir, device_ids):
            import jax

            jax.devices()
            if device_ids:
                ids = (ctypes.c_int64 * len(device_ids))(*device_ids)
                rc = lib.axon_start_nrt_profile(ids, len(device_ids))
            else:
                rc = lib.axon_start_nrt_profile(None, 0)
            if rc != 0:
                raise RuntimeError(f"axon_start_nrt_profile rc={rc}")
            try:
                yield
            finally:
                n = lib.axon_stop_nrt_profile(str(output_dir).encode())
                print(f"ntff profile: {n} file(s) -> {output_dir}",
                      file=sys.stderr)

        hooks.set_axon_ntff_profile_hook(_hook)
    except Exception:
        pass


try:
    _install_axon_ntff_hook()
except Exception:
    pass

BF16 = mybir.dt.bfloat16
F32 = mybir.dt.float32
AF = mybir.ActivationFunctionType
ALU = mybir.AluOpType
NPBF16 = ml_dtypes.bfloat16

P = 128


@dataclass(frozen=True)
class Cfg:
    T: int = 2048  # sequence length
    C: int = 1024  # input feature dim
    CO: int = 1024  # output feature dim (W_o cols)
    D: int = 64  # head dim
    HL: int = 4  # local heads per core (2 row-packed pairs)
    TQ: int = 512  # query-chunk size

    @property
    def CB(self):  # c blocks
        return self.C // P

    @property
    def NFB(self):  # qk f-blocks (q+k for HL heads)
        return 2 * self.HL * self.D // P

    @property
    def NQC(self):  # query chunks
        return self.T // self.TQ

    @property
    def TCB(self):  # t blocks of 128 (ki blocks / o_proj rows)
        return self.T // P

    @property
    def VG(self):  # v group width: [ones | pad | v] (v at partition 64)
        return self.D + 64


def emit_kernel(tc: tile.TileContext, cfg: Cfg, ins: dict, out_ap: bass.AP,
                ctx: ExitStack):
    nc = tc.nc
    T, C, CO, D, HL, TQ = cfg.T, cfg.C, cfg.CO, cfg.D, cfg.HL, cfg.TQ
    VG = cfg.VG
    CB, NQC, TCB = cfg.CB, cfg.NQC, cfg.TCB
    assert HL == 4 and D == 64 and TQ == 512

    io = ctx.enter_context(tc.tile_pool(name="io", bufs=1))

    # ---- persistent SBUF tiles + input DMA (need-ordered: wqk + x chunk 0
    # first so the first projection group can start ~6us in) ----
    warm_sb = io.tile([P, TQ], BF16, name="warm_sb", tag="warm_sb")
    nc.vector.memset(warm_sb, 0.0)
    wqk_sb = []
    for cb in range(CB):
        wq = io.tile([P, 2 * HL * D], BF16, name=f"wqk{cb}", tag=f"wqk{cb}")
        nc.sync.dma_start(wq, ins["wqk"][cb * P:(cb + 1) * P, :])
        wqk_sb.append(wq)
    xT_sb = [io.tile([P, T], BF16, name=f"xT{cb}", tag=f"xT{cb}")
             for cb in range(CB)]

    def dma_x_chunk(qc):
        for cb in range(CB):
            nc.sync.dma_start(xT_sb[cb][:, qc * TQ:(qc + 1) * TQ],
                              ins["xT"][cb * P:(cb + 1) * P,
                                        qc * TQ:(qc + 1) * TQ])

    dma_x_chunk(0)
    bbias_sb = io.tile([P, cfg.NFB], F32, name="bbias", tag="bbias")
    nc.sync.dma_start(bbias_sb, ins["bbias"][:, :])
    wv_sb = []
    for cb in range(CB):
        wvt = io.tile([P, HL * D], BF16, name=f"wv{cb}", tag=f"wv{cb}")
        nc.sync.dma_start(wvt, ins["wv"][cb * P:(cb + 1) * P, :])
        wv_sb.append(wvt)
    dma_x_chunk(1)
    wo_sb = []
    for fb in range(HL * D // P):
        wot = io.tile([P, CO], BF16, name=f"wo{fb}", tag=f"wo{fb}")
        nc.sync.dma_start(wot, ins["wo"][fb * P:(fb + 1) * P, :])
        wo_sb.append(wot)
    dma_x_chunk(2)
    dma_x_chunk(3)

    qkT_sb = [io.tile([P, T], BF16, name=f"qkT{fb}", tag=f"qkT{fb}")
              for fb in range(cfg.NFB)]
    v_all = io.tile([P, TCB * HL * VG], BF16, name="v_all", tag="v_all")
    # only the ones column (col 0 of each VG group) is ever read from the
    # non-v region; strided memset of those 64 columns is ~100x cheaper
    # than filling the whole tile
    nc.vector.memset(
        v_all.rearrange("p (g w) -> p g w", w=VG)[:, :, 0:1], 1.0)
    yT_sb = [io.tile([P, T], BF16, name=f"yT{hp}", tag=f"yT{hp}")
             for hp in range(HL // 2)]

    # PSUM pools: pp (proj/o_proj, 2 banks) + psS (4 banks) + psY (2 banks)
    pp = ctx.enter_context(tc.tile_pool(name="pp", bufs=2, space="PSUM"))
    psS = ctx.enter_context(tc.tile_pool(name="psS", bufs=1, space="PSUM"))
    psY = ctx.enter_context(tc.tile_pool(name="psY", bufs=1, space="PSUM"))
    asb = ctx.enter_context(tc.tile_pool(name="asb", bufs=3))
    osb = ctx.enter_context(tc.tile_pool(name="osb", bufs=3))

    # PE warmup during the input DMA, and early library loads for the
    # gpsimd (affine_select) / DVE (reciprocal) custom-op paths.
    # lower-triangular (qi >= ki) bf16 mask tile, built once on gpsimd;
    # the per-block causal masking is then a cheap DVE multiply instead of
    # a gpsimd affine_select (keeps masks out of the gpsimd FIFO, which
    # carries the 1us partition_broadcasts of the normalize chain)
    tri_sb = io.tile([P, P], BF16, name="tri", tag="tri")
    nc.gpsimd.memset(tri_sb, 1.0)
    nc.gpsimd.affine_select(out=tri_sb, in_=tri_sb, compare_op=ALU.is_ge,
                            fill=0.0, base=0, channel_multiplier=-1,
                            pattern=[[1, P]])
    scr2 = io.tile([1, P], F32, name="scr2", tag="scr2")
    nc.vector.memset(scr2, 1.0)
    nc.vector.reciprocal_approx_fast(scr2, scr2)
    # preload the exp ACT table set (~2.7us) during the input DMA so the
    # first real exp/Identity doesn't pay it
    nc.scalar.activation(scr2, scr2, AF.Exp)
    for w in range(12):
        wps = pp.tile([P, TQ], F32, tag="pj", name="ps_warm")
        nc.tensor.matmul(wps, warm_sb[:, 0:P], warm_sb, start=True, stop=True)

    # ---- work-group generators (PE fillers) ----
    def proj_qk_group(fb, tq):
        def emit():
            ps = pp.tile([P, TQ], F32, tag="pj", name="ps_qk")
            for cb in range(CB):
                nc.tensor.matmul(
                    ps,
                    wqk_sb[cb][:, fb * P:(fb + 1) * P],
                    xT_sb[cb][:, tq * TQ:(tq + 1) * TQ],
                    start=(cb == 0), stop=(cb == CB - 1))
            if tq == 1:
                # chunk-1 tiles are evicted during attention(0) while the
                # exp stream is still sparse; use ScalarE there to keep the
                # DVE free for the qc0->qc1 transition
                nc.scalar.activation(
                    qkT_sb[fb][:, tq * TQ:(tq + 1) * TQ], ps, AF.Identity,
                    bias=bbias_sb[:, fb:fb + 1])
            else:
                nc.vector.tensor_scalar(
                    qkT_sb[fb][:, tq * TQ:(tq + 1) * TQ], ps,
                    bbias_sb[:, fb:fb + 1], None, op0=ALU.add)
        return emit

    def proj_v_group(tb):
        def emit():
            psv = pp.tile([P, HL * D], F32, tag="pj", name="ps_v")
            for cb in range(CB):
                nc.tensor.matmul(
                    psv,
                    xT_sb[cb][:, tb * P:(tb + 1) * P],
                    wv_sb[cb],
                    start=(cb == 0), stop=(cb == CB - 1))
            # v group layout [ones | pad(63) | v]: PV then puts the softmax
            # denominator at PSUM partition 0 (custom-DVE reciprocal reads
            # base-0 PSUM correctly) and y at partition 64 (32-aligned).
            vdst = v_all[:, tb * HL * VG:(tb + 1) * HL * VG]
            vdst = vdst.rearrange("p (h g) -> p h g", g=VG)[:, :, 64:VG]
            nc.vector.tensor_copy(vdst, psv.rearrange("p (h d) -> p h d",
                                                      d=D))
        return emit

    def oproj_group(tb, jc, o_sb_box):
        def emit():
            if jc == 0:
                o_sb_box.append(osb.tile([P, CO], BF16, tag="o_sb",
                                         name="o_sb"))
            o_sb = o_sb_box[0]
            ops = pp.tile([P, TQ], F32, tag="pj", name="ps_o")
            for fb2 in range(HL * D // P):
                nc.tensor.matmul(
                    ops,
                    yT_sb[fb2][:, tb * P:(tb + 1) * P],
                    wo_sb[fb2][:, jc * TQ:(jc + 1) * TQ],
                    start=(fb2 == 0), stop=(fb2 == HL * D // P - 1))
            nc.vector.tensor_copy(o_sb[:, jc * TQ:(jc + 1) * TQ], ops)
            if jc == CO // TQ - 1:
                nc.sync.dma_start(out_ap[tb * P:(tb + 1) * P, :], o_sb)
        return emit

    fillers = deque()  # proj groups: must land before the next qc's attn
    late = deque()  # o_proj groups: anytime filler work
    for fb in (0, 2, 1, 3):
        proj_qk_group(fb, 0)()

    # ---- interleaved attention / proj / o_proj ----
    for qc in range(NQC):
        if qc + 1 < NQC:
            for fb in (0, 2, 1, 3):
                fillers.append(proj_qk_group(fb, qc + 1))
            for tb in range(4 * (qc + 1), 4 * (qc + 2)):
                fillers.append(proj_v_group(tb))
        nkb = (qc + 1) * TQ // P
        for hp in range(HL // 2):
            qtile = qkT_sb[hp]
            ktile = qkT_sb[HL // 2 + hp]
            yps = [psY.tile([P, TQ], F32, tag=f"y{h2}", name=f"ps_y{h2}")
                   for h2 in range(2)]
            def make_pv(es, ip, jj, h2):
                def emit():
                    for j2 in range(2):
                        kb = 2 * ip + j2
                        h = hp * 2 + h2
                        nc.tensor.matmul(
                            yps[h2][:, jj[j2]:TQ],
                            v_all[:, (kb * HL + h) * VG:
                                  (kb * HL + h) * VG + VG],
                            es[h2][:, j2 * TQ + jj[j2]:(j2 + 1) * TQ],
                            start=(kb == 0), stop=(kb == nkb - 1),
                            skip_group_check=True)
                return emit

            prev_pv = None
            for ip in range(nkb // 2):
                if qc == 0 and hp == 0:
                    proj_v_group(2 * ip)()
                    proj_v_group(2 * ip + 1)()
                # body order: PV(n-1,h0) + a filler first so the PE reaches
                # S(n) late (both psS tags freed by then -> the K=64 pairs
                # issue adjacently and run row-tiled concurrently);
                # PV(n-1,h1) after S(n) so a late exp_h1(n-1) doesn't let
                # the scheduler wedge PV matmuls inside the S group.
                if prev_pv is not None:
                    prev_pv[0]()
                if fillers:
                    fillers.popleft()()
                elif late:
                    late.popleft()()
                jj = [max(0, (2 * ip + j2) * P - qc * TQ) for j2 in range(2)]
                sps = [psS.tile([P, 2 * TQ], F32, tag=f"s{h2}",
                                name=f"ps_s{h2}") for h2 in range(2)]
                for j2 in range(2):
                    kb = 2 * ip + j2
                    for h2 in range(2):
                        r0, r1 = h2 * D, (h2 + 1) * D
                        # trim to causally-live query columns; the skipped
                        # PSUM region is stale but exp output there is only
                        # read by the equally-trimmed PV matmul, never used
                        nc.tensor.matmul(
                            sps[h2][:, j2 * TQ + jj[j2]:(j2 + 1) * TQ],
                            ktile[r0:r1, kb * P:(kb + 1) * P],
                            qtile[r0:r1, qc * TQ + jj[j2]:(qc + 1) * TQ],
                            start=True, stop=True)
                if prev_pv is not None:
                    prev_pv[1]()
                es = [asb.tile([P, 2 * TQ], BF16, tag=f"es{h2}",
                               name=f"es{h2}") for h2 in range(2)]
                # early (PE-bound) chunks: exp h1 first so BOTH psS tags
                # are free when the next pair's S group becomes ready ->
                # all 4 S matmuls issue adjacently and row-pair on the PE.
                # Late (ACT-bound) chunks: h0 first keeps ScalarE streaming.
                h2_order = (1, 0) if qc < 2 else (0, 1)
                for h2 in h2_order:
                    nc.scalar.activation(
                        es[h2][:, jj[0]:2 * TQ], sps[h2][:, jj[0]:2 * TQ],
                        AF.Exp)
                    for j2 in range(2):
                        kb = 2 * ip + j2
                        if kb * P >= qc * TQ:  # mask diagonal 128-square
                            dsq = es[h2][:, j2 * TQ + jj[j2]:
                                         j2 * TQ + jj[j2] + P]
                            nc.vector.tensor_tensor(dsq, dsq, tri_sb,
                                                    op=ALU.mult)
                prev_pv = (make_pv(es, ip, jj, 0), make_pv(es, ip, jj, 1))
            prev_pv[0]()
            prev_pv[1]()
            for h2 in range(2):
                # l sits at PSUM partition 0 (ones col first in v group) so
                # the custom-DVE reciprocal can read it directly.
                recip = asb.tile([1, TQ], F32, tag="recip", name="recip")
                nc.vector.reciprocal_approx_fast(recip, yps[h2][0:1, :])
                bc_sb = asb.tile([D, TQ], F32, tag="bcsb", name="bc_sb")
                nc.gpsimd.partition_broadcast(bc_sb, recip)
                nc.vector.tensor_tensor(
                    yT_sb[hp][h2 * D:(h2 + 1) * D, qc * TQ:(qc + 1) * TQ],
                    yps[h2][64:D + 64, :], bc_sb, op=ALU.mult)
        # proj(c{qc+1}) must land before attention(qc+1) starts: drain
        while fillers:
            fillers.popleft()()
        for tb in range(qc * TQ // P, (qc + 1) * TQ // P):
            box = []
            for jc in range(CO // TQ):
                late.append(oproj_group(tb, jc, box))
    while fillers:
        fillers.popleft()()
    while late:
        late.popleft()()


def build_program(cfg: Cfg, num_cores: int):
    nc = bacc.Bacc("TRN2", target_bir_lowering=False, debug=False,
                   num_devices=num_cores)
    ins = {
        "xT": nc.dram_tensor("xT", [cfg.C, cfg.T], BF16,
                             kind="ExternalInput").ap(),
        "wqk": nc.dram_tensor("wqk", [cfg.C, 2 * cfg.HL * cfg.D], BF16,
                              kind="ExternalInput").ap(),
        "wv": nc.dram_tensor("wv", [cfg.C, cfg.HL * cfg.D], BF16,
                             kind="ExternalInput").ap(),
        "wo": nc.dram_tensor("wo", [cfg.HL * cfg.D, cfg.CO], BF16,
                             kind="ExternalInput").ap(),
        "bbias": nc.dram_tensor("bbias", [P, cfg.NFB], F32,
                                kind="ExternalInput").ap(),
    }
    out_ap = nc.dram_tensor("out", [cfg.T, cfg.CO], BF16,
                            kind="ExternalOutput").ap()
    with tile.TileContext(nc) as tc:
        with ExitStack() as ctx:
            emit_kernel(tc, cfg, ins, out_ap, ctx)
    nc.compile()
    return nc


def prep_core_inputs(x_b: np.ndarray, W_qkv: np.ndarray, b_qkv: np.ndarray,
                     W_o: np.ndarray, heads, cfg: Cfg) -> dict:
    """x_b: [T, C] fp32 for this core's batch; heads: HL global head ids."""
    C, D, HL = cfg.C, cfg.D, cfg.HL
    scale = 1.0 / np.sqrt(D)
    qcols = np.concatenate([np.arange(h * D, (h + 1) * D) for h in heads])
    kcols = C + qcols
    vcols = 2 * C + qcols
    wqk = np.ascontiguousarray(
        np.concatenate([W_qkv[:, qcols] * scale, W_qkv[:, kcols]], axis=1)
    ).astype(NPBF16)
    wv = np.ascontiguousarray(W_qkv[:, vcols]).astype(NPBF16)
    wo = np.ascontiguousarray(W_o[qcols, :]).astype(NPBF16)
    bq = b_qkv[qcols].astype(np.float32)
    bk = b_qkv[kcols].astype(np.float32)
    bias_vec = np.concatenate([bq * scale, bk])
    bbias = np.ascontiguousarray(bias_vec.reshape(cfg.NFB, P).T)
    xT = np.ascontiguousarray(x_b.T).astype(NPBF16)
    return {"xT": xT, "wqk": wqk, "wv": wv, "wo": wo, "bbias": bbias}


_PROGRAM_CACHE = {}


def _get_program(cfg: Cfg, num_cores: int):
    key = (cfg, num_cores)
    if key not in _PROGRAM_CACHE:
        _PROGRAM_CACHE[key] = build_program(cfg, num_cores)
    return _PROGRAM_CACHE[key]


LAST_RESULTS = None


def kernel(x: np.ndarray, W_qkv: np.ndarray, b_qkv: np.ndarray,
           W_o: np.ndarray, b_o: np.ndarray) -> np.ndarray:
    global LAST_RESULTS
    from concourse.bass_utils import run_bass_kernel_spmd

    x = np.asarray(x, np.float32)
    W_qkv = np.asarray(W_qkv, np.float32)
    b_qkv = np.asarray(b_qkv, np.float32)
    W_o = np.asarray(W_o, np.float32)
    b_o = np.asarray(b_o, np.float32)

    B, T, C = x.shape
    H = 16
    cfg = Cfg(T=T, C=C, CO=W_o.shape[1], D=C // H, HL=4)
    n_cores = 8
    groups = H // cfg.HL  # 4 head groups
    assert B * groups == n_cores

    nc = _get_program(cfg, n_cores)

    in_maps = []
    for core in range(n_cores):
        b, hg = core // groups, core % groups
        heads = list(range(hg * cfg.HL, (hg + 1) * cfg.HL))
        in_maps.append(prep_core_inputs(x[b], W_qkv, b_qkv, W_o, heads, cfg))

    res = run_bass_kernel_spmd(nc, in_maps, core_ids=list(range(n_cores)))
    LAST_RESULTS = res

    out = np.zeros((B, T, cfg.CO), np.float32)
    for core in range(n_cores):
        out[core // groups] += np.asarray(res.results[core]["out"],
                                          dtype=np.float32)
    # softmax rows sum to 1, so the v-bias contributes b_v @ W_o to every
    # output row; fold it into the output bias on the host.
    bias_full = b_o + b_qkv[2 * C:3 * C] @ W_o
    out += bias_full[None, None, :].astype(np.float32)
    return out


# revision 60
# speedup vs baseline: 1.0148x; 1.0148x over previous
"""Causal multi-head attention forward (B=2, T=2048, C=1024, H=16, D=64)
for 8 Trainium2 NeuronCores.

Sharding: core = (batch b, head-group hg) with b in {0,1}, hg in {0..3};
each core computes QKV projection for its 4 heads on its batch, causal
flash attention for those heads, and a partial output projection
(contraction over its 256 head-feature rows of W_o). Host sums the 4
partials per batch and adds b_o.

Schedule: projection, attention and o_proj are interleaved per query
chunk so the PE never drains while ScalarE runs the big exp stream
(keeps the HAM clock-gate released):

  proj(c0); for qc: [attention(qc) pairs, with proj(c{qc+1}) groups and
  deferred o_proj groups used as PE fillers between pairs]; o_proj(qc)
  groups are queued jc-granular and consumed late.

Per kb-pair (2 key blocks x 2 heads of the pair):
  - S matmuls (K=64) use auto-derived tile_position (0,0)/(64,0) so
    adjacent-issued pairs run concurrently on disjoint PE row halves;
    for the PE-bound early chunks (qc<2) the exps are ordered h1-first
    so both psS tags free together and the next S quad issues
    back-to-back (for ACT-bound late chunks h0-first keeps ScalarE
    streaming continuously instead).
  - S and PV are column-trimmed to the causally live range [jj:512];
    exp covers [jj0:1024] per (pair, h2) in one ScalarE instruction
    (the stale gap columns it exps are never read back).
  - the causal diagonal 128-square is masked by a DVE multiply with a
    precomputed triangular bf16 tile (keeps the gpsimd FIFO, which
    carries the 1us normalize broadcasts, off the critical path).
  - PV(pair n) is emitted around S(pair n+1): h2=0 before the filler
    group, h2=1 after the S quad, so a late exp cannot let the
    scheduler wedge full-row PV matmuls inside the S quad.

Kernel-internal layouts (per core):
  xT    [C, T]    bf16   x transposed (host-prepped)
  wqk   [C, 512]  bf16   [q cols heads0..3 | k cols heads0..3], q
                         pre-scaled by 1/sqrt(D) host-side
  wv    [C, 256]  bf16
  wo    [256, CO] bf16   W_o rows for this head group
  qkT   [512, T]  bf16   biases added at eviction (DVE, per-partition)
  S^T   [ki, qi]  PSUM   scores transposed, 2 kb blocks per tile
  expS  [ki, qi]  bf16   exp on ScalarE
  yT'   [128, qi] PSUM   [ones|pad|v]^T @ expS -> row 0 = denominator,
                         rows 64..127 = unnormalized y^T
  norm: recip(l) -> gpsimd partition_broadcast -> DVE multiply
  out   [T, CO]   bf16   partial o_proj; host sums partials in fp32
"""

import os
import sys
from collections import deque
from contextlib import ExitStack
from dataclasses import dataclass

import numpy as np

for _p in ("/opt/trn_rl_repo",):
    if _p not in sys.path and os.path.isdir(_p):
        sys.path.insert(0, _p)

import ml_dtypes

import concourse.bass as bass
import concourse.bacc as bacc
import concourse.mybir as mybir
import concourse.tile as tile


def _install_axon_ntff_hook():
    """Provide antenv.axon_hooks (absent on this image) so bass_utils'
    trace path works; registers the ctypes NTFF hook when available."""
    import types

    if "antenv.axon_hooks" not in sys.modules:
        import antenv

        mod = types.ModuleType("antenv.axon_hooks")
        _reg = [None]
        mod.get_axon_ntff_profile_hook = lambda: _reg[0]
        mod.set_axon_ntff_profile_hook = lambda h: _reg.__setitem__(0, h)
        sys.modules["antenv.axon_hooks"] = mod
        antenv.axon_hooks = mod
    hooks = sys.modules["antenv.axon_hooks"]
    if hooks.get_axon_ntff_profile_hook() is not None:
        return
    try:
        import contextlib
        import ctypes

        lib = ctypes.CDLL("/opt/axon/libaxon_pjrt.so")
        if not hasattr(lib, "axon_start_nrt_profile"):
            return
        lib.axon_start_nrt_profile.argtypes = [
            ctypes.POINTER(ctypes.c_int64), ctypes.c_size_t]
        lib.axon_start_nrt_profile.restype = ctypes.c_int64
        lib.axon_stop_nrt_profile.argtypes = [ctypes.c_char_p]
        lib.axon_stop_nrt_profile.restype = ctypes.c_int64

        @contextlib.contextmanager
        def _hook(output_dir, device_ids):
            import jax

            jax.devices()
            if device_ids:
                ids = (ctypes.c_int64 * len(device_ids))(*device_ids)
                rc = lib.axon_start_nrt_profile(ids, len(device_ids))
            else:
                rc = lib.axon_start_nrt_profile(None, 0)
            if rc != 0:
                raise RuntimeError(f"axon_start_nrt_profile rc={rc}")
            try:
                yield
            finally:
                n = lib.axon_stop_nrt_profile(str(output_dir).encode())
                print(f"ntff profile: {n} file(s) -> {output_dir}",
                      file=sys.stderr)

        hooks.set_axon_ntff_profile_hook(_hook)
    except Exception:
        pass


try:
    _install_axon_ntff_hook()
except Exception:
    pass

BF16 = mybir.dt.bfloat16
F32 = mybir.dt.float32
AF = mybir.ActivationFunctionType
ALU = mybir.AluOpType
NPBF16 = ml_dtypes.bfloat16

P = 128


@dataclass(frozen=True)
class Cfg:
    T: int = 2048  # sequence length
    C: int = 1024  # input feature dim
    CO: int = 1024  # output feature dim (W_o cols)
    D: int = 64  # head dim
    HL: int = 4  # local heads per core (2 row-packed pairs)
    TQ: int = 512  # query-chunk size

    @property
    def CB(self):  # c blocks
        return self.C // P

    @property
    def NFB(self):  # qk f-blocks (q+k for HL heads)
        return 2 * self.HL * self.D // P

    @property
    def NQC(self):  # query chunks
        return self.T // self.TQ

    @property
    def TCB(self):  # t blocks of 128 (ki blocks / o_proj rows)
        return self.T // P

    @property
    def VG(self):  # v group width: [ones | pad | v] (v at partition 64)
        return self.D + 64


def emit_kernel(tc: tile.TileContext, cfg: Cfg, ins: dict, out_ap: bass.AP,
                ctx: ExitStack):
    nc = tc.nc
    T, C, CO, D, HL, TQ = cfg.T, cfg.C, cfg.CO, cfg.D, cfg.HL, cfg.TQ
    VG = cfg.VG
    CB, NQC, TCB = cfg.CB, cfg.NQC, cfg.TCB
    assert HL == 4 and D == 64 and TQ == 512

    io = ctx.enter_context(tc.tile_pool(name="io", bufs=1))

    # ---- persistent SBUF tiles + input DMA (need-ordered: wqk + x chunk 0
    # first so the first projection group can start ~6us in) ----
    warm_sb = io.tile([P, TQ], BF16, name="warm_sb", tag="warm_sb")
    nc.vector.memset(warm_sb, 0.0)
    wqk_sb = []
    for cb in range(CB):
        wq = io.tile([P, 2 * HL * D], BF16, name=f"wqk{cb}", tag=f"wqk{cb}")
        nc.sync.dma_start(wq, ins["wqk"][cb * P:(cb + 1) * P, :])
        wqk_sb.append(wq)
    xT_sb = [io.tile([P, T], BF16, name=f"xT{cb}", tag=f"xT{cb}")
             for cb in range(CB)]

    def dma_x_chunk(qc):
        for cb in range(CB):
            nc.sync.dma_start(xT_sb[cb][:, qc * TQ:(qc + 1) * TQ],
                              ins["xT"][cb * P:(cb + 1) * P,
                                        qc * TQ:(qc + 1) * TQ])

    dma_x_chunk(0)
    bbias_sb = io.tile([P, cfg.NFB], F32, name="bbias", tag="bbias")
    nc.sync.dma_start(bbias_sb, ins["bbias"][:, :])
    wv_sb = []
    for cb in range(CB):
        wvt = io.tile([P, HL * D], BF16, name=f"wv{cb}", tag=f"wv{cb}")
        nc.sync.dma_start(wvt, ins["wv"][cb * P:(cb + 1) * P, :])
        wv_sb.append(wvt)
    dma_x_chunk(1)
    wo_sb = []
    for fb in range(HL * D // P):
        wot = io.tile([P, CO], BF16, name=f"wo{fb}", tag=f"wo{fb}")
        nc.sync.dma_start(wot, ins["wo"][fb * P:(fb + 1) * P, :])
        wo_sb.append(wot)
    dma_x_chunk(2)
    dma_x_chunk(3)

    qkT_sb = [io.tile([P, T], BF16, name=f"qkT{fb}", tag=f"qkT{fb}")
              for fb in range(cfg.NFB)]
    v_all = io.tile([P, TCB * HL * VG], BF16, name="v_all", tag="v_all")
    # only the ones column (col 0 of each VG group) is ever read from the
    # non-v region; strided memset of those 64 columns is ~100x cheaper
    # than filling the whole tile
    nc.vector.memset(
        v_all.rearrange("p (g w) -> p g w", w=VG)[:, :, 0:1], 1.0)
    yT_sb = [io.tile([P, T], BF16, name=f"yT{hp}", tag=f"yT{hp}")
             for hp in range(HL // 2)]

    # PSUM pools: pp (proj/o_proj, 2 banks) + psS (4 banks) + psY (2 banks)
    pp = ctx.enter_context(tc.tile_pool(name="pp", bufs=2, space="PSUM"))
    psS = ctx.enter_context(tc.tile_pool(name="psS", bufs=1, space="PSUM"))
    psY = ctx.enter_context(tc.tile_pool(name="psY", bufs=1, space="PSUM"))
    asb = ctx.enter_context(tc.tile_pool(name="asb", bufs=3))
    osb = ctx.enter_context(tc.tile_pool(name="osb", bufs=3))

    # PE warmup during the input DMA, and early library loads for the
    # gpsimd (affine_select) / DVE (reciprocal) custom-op paths.
    # lower-triangular (qi >= ki) bf16 mask tile, built once on gpsimd;
    # the per-block causal masking is then a cheap DVE multiply instead of
    # a gpsimd affine_select (keeps masks out of the gpsimd FIFO, which
    # carries the 1us partition_broadcasts of the normalize chain)
    tri_sb = io.tile([P, P], BF16, name="tri", tag="tri")
    nc.gpsimd.memset(tri_sb, 1.0)
    nc.gpsimd.affine_select(out=tri_sb, in_=tri_sb, compare_op=ALU.is_ge,
                            fill=0.0, base=0, channel_multiplier=-1,
                            pattern=[[1, P]])
    scr2 = io.tile([1, P], F32, name="scr2", tag="scr2")
    nc.vector.memset(scr2, 1.0)
    nc.vector.reciprocal_approx_fast(scr2, scr2)
    # preload the exp ACT table set (~2.7us) during the input DMA so the
    # first real exp/Identity doesn't pay it
    nc.scalar.activation(scr2, scr2, AF.Exp)
    for w in range(12):
        wps = pp.tile([P, TQ], F32, tag="pj", name="ps_warm")
        nc.tensor.matmul(wps, warm_sb[:, 0:P], warm_sb, start=True, stop=True)

    # ---- work-group generators (PE fillers) ----
    def proj_qk_group(fb, tq):
        def emit():
            ps = pp.tile([P, TQ], F32, tag="pj", name="ps_qk")
            for cb in range(CB):
                nc.tensor.matmul(
                    ps,
                    wqk_sb[cb][:, fb * P:(fb + 1) * P],
                    xT_sb[cb][:, tq * TQ:(tq + 1) * TQ],
                    start=(cb == 0), stop=(cb == CB - 1))
            nc.vector.tensor_scalar(
                qkT_sb[fb][:, tq * TQ:(tq + 1) * TQ], ps,
                bbias_sb[:, fb:fb + 1], None, op0=ALU.add)
        return emit

    def proj_v_group(tb):
        def emit():
            psv = pp.tile([P, HL * D], F32, tag="pj", name="ps_v")
            for cb in range(CB):
                nc.tensor.matmul(
                    psv,
                    xT_sb[cb][:, tb * P:(tb + 1) * P],
                    wv_sb[cb],
                    start=(cb == 0), stop=(cb == CB - 1))
            # v group layout [ones | pad(63) | v]: PV then puts the softmax
            # denominator at PSUM partition 0 (custom-DVE reciprocal reads
            # base-0 PSUM correctly) and y at partition 64 (32-aligned).
            vdst = v_all[:, tb * HL * VG:(tb + 1) * HL * VG]
            vdst = vdst.rearrange("p (h g) -> p h g", g=VG)[:, :, 64:VG]
            nc.vector.tensor_copy(vdst, psv.rearrange("p (h d) -> p h d",
                                                      d=D))
        return emit

    def oproj_group(tb, jc, o_sb_box):
        def emit():
            if jc == 0:
                o_sb_box.append(osb.tile([P, CO], BF16, tag="o_sb",
                                         name="o_sb"))
            o_sb = o_sb_box[0]
            ops = pp.tile([P, TQ], F32, tag="pj", name="ps_o")
            for fb2 in range(HL * D // P):
                nc.tensor.matmul(
                    ops,
                    yT_sb[fb2][:, tb * P:(tb + 1) * P],
                    wo_sb[fb2][:, jc * TQ:(jc + 1) * TQ],
                    start=(fb2 == 0), stop=(fb2 == HL * D // P - 1))
            nc.vector.tensor_copy(o_sb[:, jc * TQ:(jc + 1) * TQ], ops)
            if jc == CO // TQ - 1:
                nc.sync.dma_start(out_ap[tb * P:(tb + 1) * P, :], o_sb)
        return emit

    fillers = deque()  # proj groups: must land before the next qc's attn
    late = deque()  # o_proj groups: anytime filler work
    for fb in (0, 2, 1, 3):
        proj_qk_group(fb, 0)()

    # ---- interleaved attention / proj / o_proj ----
    for qc in range(NQC):
        if qc + 1 < NQC:
            for fb in (0, 2, 1, 3):
                fillers.append(proj_qk_group(fb, qc + 1))
            for tb in range(4 * (qc + 1), 4 * (qc + 2)):
                fillers.append(proj_v_group(tb))
        nkb = (qc + 1) * TQ // P
        for hp in range(HL // 2):
            qtile = qkT_sb[hp]
            ktile = qkT_sb[HL // 2 + hp]
            yps = [psY.tile([P, TQ], F32, tag=f"y{h2}", name=f"ps_y{h2}")
                   for h2 in range(2)]
            def make_pv(es, ip, jj, h2):
                def emit():
                    for j2 in range(2):
                        kb = 2 * ip + j2
                        h = hp * 2 + h2
                        nc.tensor.matmul(
                            yps[h2][:, jj[j2]:TQ],
                            v_all[:, (kb * HL + h) * VG:
                                  (kb * HL + h) * VG + VG],
                            es[h2][:, j2 * TQ + jj[j2]:(j2 + 1) * TQ],
                            start=(kb == 0), stop=(kb == nkb - 1),
                            skip_group_check=True)
                return emit

            prev_pv = None
            for ip in range(nkb // 2):
                if qc == 0 and hp == 0:
                    proj_v_group(2 * ip)()
                    proj_v_group(2 * ip + 1)()
                # body order: PV(n-1,h0) + a filler first so the PE reaches
                # S(n) late (both psS tags freed by then -> the K=64 pairs
                # issue adjacently and run row-tiled concurrently);
                # PV(n-1,h1) after S(n) so a late exp_h1(n-1) doesn't let
                # the scheduler wedge PV matmuls inside the S group.
                if prev_pv is not None:
                    prev_pv[0]()
                if fillers:
                    fillers.popleft()()
                elif late:
                    late.popleft()()
                jj = [max(0, (2 * ip + j2) * P - qc * TQ) for j2 in range(2)]
                sps = [psS.tile([P, 2 * TQ], F32, tag=f"s{h2}",
                                name=f"ps_s{h2}") for h2 in range(2)]
                for j2 in range(2):
                    kb = 2 * ip + j2
                    for h2 in range(2):
                        r0, r1 = h2 * D, (h2 + 1) * D
                        # trim to causally-live query columns; the skipped
                        # PSUM region is stale but exp output there is only
                        # read by the equally-trimmed PV matmul, never used
                        nc.tensor.matmul(
                            sps[h2][:, j2 * TQ + jj[j2]:(j2 + 1) * TQ],
                            ktile[r0:r1, kb * P:(kb + 1) * P],
                            qtile[r0:r1, qc * TQ + jj[j2]:(qc + 1) * TQ],
                            start=True, stop=True)
                if prev_pv is not None:
                    prev_pv[1]()
                es = [asb.tile([P, 2 * TQ], BF16, tag=f"es{h2}",
                               name=f"es{h2}") for h2 in range(2)]
                # early (PE-bound) chunks: exp h1 first so BOTH psS tags
                # are free when the next pair's S group becomes ready ->
                # all 4 S matmuls issue adjacently and row-pair on the PE.
                # Late (ACT-bound) chunks: h0 first keeps ScalarE streaming.
                h2_order = (1, 0) if qc < 2 else (0, 1)
                for h2 in h2_order:
                    nc.scalar.activation(
                        es[h2][:, jj[0]:2 * TQ], sps[h2][:, jj[0]:2 * TQ],
                        AF.Exp)
                    for j2 in range(2):
                        kb = 2 * ip + j2
                        if kb * P >= qc * TQ:  # mask diagonal 128-square
                            dsq = es[h2][:, j2 * TQ + jj[j2]:
                                         j2 * TQ + jj[j2] + P]
                            nc.vector.tensor_tensor(dsq, dsq, tri_sb,
                                                    op=ALU.mult)
                prev_pv = (make_pv(es, ip, jj, 0), make_pv(es, ip, jj, 1))
            prev_pv[0]()
            prev_pv[1]()
            for h2 in range(2):
                # l sits at PSUM partition 0 (ones col first in v group) so
                # the custom-DVE reciprocal can read it directly.
                recip = asb.tile([1, TQ], F32, tag="recip", name="recip")
                nc.vector.reciprocal_approx_fast(recip, yps[h2][0:1, :])
                bc_sb = asb.tile([D, TQ], F32, tag="bcsb", name="bc_sb")
                nc.gpsimd.partition_broadcast(bc_sb, recip)
                nc.vector.tensor_tensor(
                    yT_sb[hp][h2 * D:(h2 + 1) * D, qc * TQ:(qc + 1) * TQ],
                    yps[h2][64:D + 64, :], bc_sb, op=ALU.mult)
        # proj(c{qc+1}) must land before attention(qc+1) starts: drain
        while fillers:
            fillers.popleft()()
        for tb in range(qc * TQ // P, (qc + 1) * TQ // P):
            box = []
            for jc in range(CO // TQ):
                late.append(oproj_group(tb, jc, box))
    while fillers:
        fillers.popleft()()
    while late:
        late.popleft()()


def build_program(cfg: Cfg, num_cores: int):
    nc = bacc.Bacc("TRN2", target_bir_lowering=False, debug=False,
                   num_devices=num_cores)
    ins = {
        "xT": nc.dram_tensor("xT", [cfg.C, cfg.T], BF16,
                             kind="ExternalInput").ap(),
        "wqk": nc.dram_tensor("wqk", [cfg.C, 2 * cfg.HL * cfg.D], BF16,
                              kind="ExternalInput").ap(),
        "wv": nc.dram_tensor("wv", [cfg.C, cfg.HL * cfg.D], BF16,
                             kind="ExternalInput").ap(),
        "wo": nc.dram_tensor("wo", [cfg.HL * cfg.D, cfg.CO], BF16,
                             kind="ExternalInput").ap(),
        "bbias": nc.dram_tensor("bbias", [P, cfg.NFB], F32,
                                kind="ExternalInput").ap(),
    }
    out_ap = nc.dram_tensor("out", [cfg.T, cfg.CO], BF16,
                            kind="ExternalOutput").ap()
    with tile.TileContext(nc) as tc:
        with ExitStack() as ctx:
            emit_kernel(tc, cfg, ins, out_ap, ctx)
    nc.compile()
    return nc


def prep_core_inputs(x_b: np.ndarray, W_qkv: np.ndarray, b_qkv: np.ndarray,
                     W_o: np.ndarray, heads, cfg: Cfg) -> dict:
    """x_b: [T, C] fp32 for this core's batch; heads: HL global head ids."""
    C, D, HL = cfg.C, cfg.D, cfg.HL
    scale = 1.0 / np.sqrt(D)
    qcols = np.concatenate([np.arange(h * D, (h + 1) * D) for h in heads])
    kcols = C + qcols
    vcols = 2 * C + qcols
    wqk = np.ascontiguousarray(
        np.concatenate([W_qkv[:, qcols] * scale, W_qkv[:, kcols]], axis=1)
    ).astype(NPBF16)
    wv = np.ascontiguousarray(W_qkv[:, vcols]).astype(NPBF16)
    wo = np.ascontiguousarray(W_o[qcols, :]).astype(NPBF16)
    bq = b_qkv[qcols].astype(np.float32)
    bk = b_qkv[kcols].astype(np.float32)
    bias_vec = np.concatenate([bq * scale, bk])
    bbias = np.ascontiguousarray(bias_vec.reshape(cfg.NFB, P).T)
    xT = np.ascontiguousarray(x_b.T).astype(NPBF16)
    return {"xT": xT, "wqk": wqk, "wv": wv, "wo": wo, "bbias": bbias}


_PROGRAM_CACHE = {}


def _get_program(cfg: Cfg, num_cores: int):
    key = (cfg, num_cores)
    if key not in _PROGRAM_CACHE:
        _PROGRAM_CACHE[key] = build_program(cfg, num_cores)
    return _PROGRAM_CACHE[key]


LAST_RESULTS = None


def kernel(x: np.ndarray, W_qkv: np.ndarray, b_qkv: np.ndarray,
           W_o: np.ndarray, b_o: np.ndarray) -> np.ndarray:
    global LAST_RESULTS
    from concourse.bass_utils import run_bass_kernel_spmd

    x = np.asarray(x, np.float32)
    W_qkv = np.asarray(W_qkv, np.float32)
    b_qkv = np.asarray(b_qkv, np.float32)
    W_o = np.asarray(W_o, np.float32)
    b_o = np.asarray(b_o, np.float32)

    B, T, C = x.shape
    H = 16
    cfg = Cfg(T=T, C=C, CO=W_o.shape[1], D=C // H, HL=4)
    n_cores = 8
    groups = H // cfg.HL  # 4 head groups
    assert B * groups == n_cores

    nc = _get_program(cfg, n_cores)

    in_maps = []
    for core in range(n_cores):
        b, hg = core // groups, core % groups
        heads = list(range(hg * cfg.HL, (hg + 1) * cfg.HL))
        in_maps.append(prep_core_inputs(x[b], W_qkv, b_qkv, W_o, heads, cfg))

    res = run_bass_kernel_spmd(nc, in_maps, core_ids=list(range(n_cores)))
    LAST_RESULTS = res

    out = np.zeros((B, T, cfg.CO), np.float32)
    for core in range(n_cores):
        out[core // groups] += np.asarray(res.results[core]["out"],
                                          dtype=np.float32)
    # softmax rows sum to 1, so the v-bias contributes b_v @ W_o to every
    # output row; fold it into the output bias on the host.
    bias_full = b_o + b_qkv[2 * C:3 * C] @ W_o
    out += bias_full[None, None, :].astype(np.float32)
    return out
